# revision 52
# baseline (speedup 1.0000x reference)
"""Trainium2 Bass/Tile kernel for nn_EncoderLayer (dense transformer block).

Strategy: pure data-parallel over batch (B=8 -> 1 batch element per core, no
collectives). Per core, activations are kept feature-major ([D, T]) in bf16
(same PE matmul rate as fp32r, half the DMA/SBUF traffic, 2x DVE). The key
mask folds into the exp bias (per-partition = per-key) so V needs no masking
and a constant ones column appended to V yields the softmax normalizer for
free from the same P@V matmuls. The attention inner loop is software-
pipelined with the score lookahead running across head boundaries, keeping
the Activation engine's exp stream (the bottleneck of that phase) saturated;
1/norm rows are partition-replicated with PE outer products and applied on
the DVE. h1 stays resident in SBUF (no DRAM round trip). FFN1 output (all 40
row-tiles) stays resident in bf16 so FFN2 accumulates entirely in PSUM with
no SBUF accumulation adds. FFN2/LayerNorm2/transpose/store are pipelined
over T-halves: the finished half's LN2 statistics (computed row-major via
1-column PE matmuls), normalize, PE-transpose to row-major, and DMA-out all
ride under the other half's PE sweep. LayerNorm statistic chains run on
[128, 8] row-major tiles (13x cheaper than [1, T] rows on the DVE).
Startup DMAs are interleaved in consumption order; activation tables (Exp,
Sqrt) are preloaded off the critical path with dummy ops.
"""

import json
import sys

if "/opt/trn_rl_repo" not in sys.path:
    sys.path.insert(0, "/opt/trn_rl_repo")

import numpy as np
import ml_dtypes

import concourse.bass as bass
import concourse.mybir as mybir
import concourse.tile as tile

B, T, CC, DM, H, DH, DFF, K = 8, 1024, 256, 1024, 16, 64, 5120, 3
EMB = CC + DM  # 1280
EPS = 1e-6
f32 = mybir.dt.float32
bf16 = mybir.dt.bfloat16
AF = mybir.ActivationFunctionType
OP = mybir.AluOpType

NT = T // 128          # 8 time tiles
NKE = EMB // 128       # 10 embed k-tiles
NKD = DM // 128        # 8 d_model k-tiles
NMF = DFF // 128       # 40 d_ff tiles
HV = DH + 1            # 65: per-head V columns + normalizer ones column
MASK_NEG = -60000.0    # exp(-60000 + s/8) == 0.0 in f32


def _mm(nc, out, lhsT, rhs, start, stop):
    nc.tensor.matmul(out, lhsT, rhs, start=start, stop=stop)


def _ln_factors(nc, pool, mmpool, stat, c0, ntc, seqP, ident, ones_row,
                epsP, sfx):
    """From row-major PSUM sums stat[:, c0:c0+ntc]=sum(x) and
    stat[:, 8+c0:...]=sum(x^2) (indexed by (t%128, t//128)), produce
    muF/rsF [128, ntc*128] bf16 partition-replicated tiles. Elementwise work
    is on [128, ntc] tiles; rows come from 1-column PE transposes; the
    replication is a PE outer product with ones_row."""
    f32 = mybir.dt.float32
    bf16 = mybir.dt.bfloat16
    AF = mybir.ActivationFunctionType
    OP = mybir.AluOpType
    W = ntc * 128
    sc = pool.tile([128, 2, ntc], f32, tag="mur" + sfx)
    nc.vector.tensor_scalar(
        sc[:], stat.rearrange("p (a c) -> p a c", a=2)[:, :, c0:c0 + ntc],
        1.0 / EMB, EPS, OP.mult, OP.add)
    mur = sc[:, 0, :]
    ex2r = sc[:, 1, :]
    mu2r = pool.tile([128, ntc], f32, tag="mu2r" + sfx)
    nc.scalar.activation(mu2r[:], mur, AF.Square)
    varr = pool.tile([128, ntc], f32, tag="varr" + sfx)
    nc.vector.tensor_sub(varr[:], ex2r, mu2r[:])  # includes +eps
    vrecr = pool.tile([128, ntc], f32, tag="mu2r" + sfx)
    nc.vector.reciprocal(vrecr[:], varr[:])
    rsr = pool.tile([128, ntc], f32, tag="ex2r" + sfx)
    nc.scalar.activation(rsr[:], vrecr[:], AF.Sqrt)
    muB = pool.tile([128, ntc], bf16, tag="muB" + sfx)
    with nc.allow_low_precision(reason="bf16 LN factors"):
        nc.vector.tensor_copy(muB[:], mur)
    rsB = pool.tile([128, ntc], bf16, tag="rsB" + sfx)
    with nc.allow_low_precision(reason="bf16 LN factors"):
        nc.vector.tensor_mul(rsB[:], rsr[:], seqP[:, c0:c0 + ntc])
    muF = pool.tile([128, W], bf16, tag="muF" + sfx)
    rsF = pool.tile([128, W], bf16, tag="rsF" + sfx)
    for src_, dst in ((muB, muF), (rsB, rsF)):
        rowp = mmpool.tile([1, W], bf16, tag="lnbc", bufs=2)
        for i in range(ntc):
            nc.tensor.matmul(rowp[:, i * 128:(i + 1) * 128],
                             src_[:, i:i + 1], ident[:],
                             start=True, stop=True, is_transpose=True)
        srow = pool.tile([1, W], bf16, tag="srow" + ("m" if dst is muF else "r") + sfx)
        nc.scalar.activation(srow[:], rowp[:], AF.Identity)
        for c in range(W // 512):
            psb = mmpool.tile([128, 512], f32, tag="lnbc", bufs=2)
            nc.tensor.matmul(psb[:], ones_row[:],
                             srow[:, c * 512:(c + 1) * 512],
                             start=True, stop=True)
            if c % 2 == 0:
                nc.scalar.activation(dst[:, c * 512:(c + 1) * 512], psb[:],
                                     AF.Identity)
            else:
                nc.vector.tensor_copy(dst[:, c * 512:(c + 1) * 512], psb[:])
    return muF, rsF


def build_nc():
    nc = bass.Bass()

    xt_d = nc.declare_dram_parameter("xt", [EMB, T], bf16, isOutput=False)
    wv_d = nc.declare_dram_parameter("wv", [DM, DM], bf16, isOutput=False)
    wqr_d = nc.declare_dram_parameter("wqr", [8, 128, 8, 128], bf16, isOutput=False)
    wkr_d = nc.declare_dram_parameter("wkr", [8, 128, 8, 128], bf16, isOutput=False)
    wor_d = nc.declare_dram_parameter("wor", [8, 128, 8, 128], bf16, isOutput=False)
    w1r_d = nc.declare_dram_parameter("w1r", [40, 128, 10, 128], bf16, isOutput=False)
    w2r_d = nc.declare_dram_parameter("w2r", [10, 128, 40, 128], bf16, isOutput=False)
    bvf_d = nc.declare_dram_parameter("bvf", [128, DM], bf16, isOutput=False)
    mbias_d = nc.declare_dram_parameter("mbias", [128, 8], f32, isOutput=False)
    bqp_d = nc.declare_dram_parameter("bqp", [128, 8], f32, isOutput=False)
    bkp_d = nc.declare_dram_parameter("bkp", [128, 8], f32, isOutput=False)
    bop_d = nc.declare_dram_parameter("bop", [128, 8], f32, isOutput=False)
    b1p_d = nc.declare_dram_parameter("b1p", [128, 40], f32, isOutput=False)
    b2p_d = nc.declare_dram_parameter("b2p", [128, 10], f32, isOutput=False)
    g1p_d = nc.declare_dram_parameter("g1p", [128, 10], f32, isOutput=False)
    beta1p_d = nc.declare_dram_parameter("beta1p", [128, 10], f32, isOutput=False)
    g2p_d = nc.declare_dram_parameter("g2p", [128, 10], f32, isOutput=False)
    beta2p_d = nc.declare_dram_parameter("beta2p", [128, 10], f32, isOutput=False)
    cwbc_d = nc.declare_dram_parameter("cwbc", [128, K], f32, isOutput=False)
    seqp_d = nc.declare_dram_parameter("seqp", [128, 8], f32, isOutput=False)
    onescol_d = nc.declare_dram_parameter("onescol", [128, 1], bf16, isOutput=False)
    onesrow_d = nc.declare_dram_parameter("onesrow", [1, 128], bf16, isOutput=False)
    ident_d = nc.declare_dram_parameter("ident", [128, 128], bf16, isOutput=False)
    out_d = nc.declare_dram_parameter("out", [T, EMB], f32, isOutput=True)

    with tile.TileContext(nc) as tc:
        # ---------------- persistent pools (alloc in reverse-death order) ---
        constp = tc.alloc_tile_pool(name="constp", bufs=1)
        h1p = tc.alloc_tile_pool(name="h1p", bufs=1)
        h1 = h1p.tile([128, NKE, T], bf16)
        h1prep = tc.alloc_tile_pool(name="h1prep", bufs=1)
        h1pre = h1prep.tile([128, NKE, T], bf16)
        attp = tc.alloc_tile_pool(name="attp", bufs=1)
        attT = attp.tile([128, NKD, T], bf16)
        xtp = tc.alloc_tile_pool(name="xtp", bufs=1)
        xt = xtp.tile([128, NKE, T], bf16)

        # const tiles allocated now; DMAs deferred past the startup stream
        bvF = constp.tile([128, DM], bf16)
        mbias = constp.tile([128, 8], f32)
        bqP = constp.tile([128, 8], f32)
        bkP = constp.tile([128, 8], f32)
        boP = constp.tile([128, 8], f32)
        b1P = constp.tile([128, 40], f32)
        b2P = constp.tile([128, 10], f32)
        g1P = constp.tile([128, 10], f32)
        beta1P = constp.tile([128, 10], f32)
        g2P = constp.tile([128, 10], f32)
        beta2P = constp.tile([128, 10], f32)
        cwbc = constp.tile([128, K], f32)
        seqP = constp.tile([128, 8], f32)
        ones_col = constp.tile([128, 1], bf16)
        ones_row = constp.tile([1, 128], bf16)
        ident = constp.tile([128, 128], bf16)
        epsP = constp.tile([128, 1], f32)
        nc.gpsimd.memset(epsP[:], EPS)
        dumt = constp.tile([1, 2], f32)
        NSTAGE = 6
        w1stage = [constp.tile([128, 10, 128], bf16, name=f"w1s{i}")
                   for i in range(NSTAGE)]

        def emit_const_dmas():
            nc.sync.dma_start(mbias[:], mbias_d[:])
            nc.sync.dma_start(bqP[:], bqp_d[:])
            nc.sync.dma_start(bkP[:], bkp_d[:])
            nc.sync.dma_start(boP[:], bop_d[:])
            nc.sync.dma_start(b1P[:], b1p_d[:])
            nc.sync.dma_start(b2P[:], b2p_d[:])
            nc.sync.dma_start(g1P[:], g1p_d[:])
            nc.sync.dma_start(beta1P[:], beta1p_d[:])
            nc.sync.dma_start(g2P[:], g2p_d[:])
            nc.sync.dma_start(beta2P[:], beta2p_d[:])
            nc.sync.dma_start(cwbc[:], cwbc_d[:])
            nc.sync.dma_start(seqP[:], seqp_d[:])
            nc.sync.dma_start(ones_col[:], onescol_d[:])
            nc.sync.dma_start(ones_row[:], onesrow_d[:])
            nc.sync.dma_start(ident[:], ident_d[:])

        vp = tc.alloc_tile_pool(name="vp", bufs=1)
        vaug = vp.tile([128, NT, H * HV], bf16)
        # normalizer ones column (col DH of each head slot)
        ocols = vaug.rearrange("p j (h c) -> p (j h) c", c=HV)[:, :, DH:HV]
        nc.gpsimd.memset(ocols, 1.0)

        qkp = tc.alloc_tile_pool(name="qkp", bufs=1)
        qt = qkp.tile([128, NKD, T], bf16)
        kt = qkp.tile([128, NKD, T], bf16)

        with tc.tile_pool(name="wqp", bufs=3) as wqp:
            qk_wts = []

            def emit_qk_load(i):
                wdram = wqr_d if i < 8 else wkr_d
                wt = wqp.tile([128, 8, 128], bf16, tag="wt")
                nc.sync.dma_start(wt[:], wdram[i % 8])
                qk_wts.append(wt)

            # ------------ V projection (row-major, bias, augmented) --------
            with (
                tc.tile_pool(name="wvp", bufs=4) as wvp,
                tc.tile_pool(name="vtmp", bufs=4) as vtmp,
                tc.tile_pool(name="vps", bufs=8, space="PSUM") as vps,
            ):
                for n in range(2):
                    pss = [vps.tile([128, 512], f32, name=f"vps{i}", tag="vps")
                           for i in range(NT)]
                    for k in range(NKD):
                        if n == 0:  # startup: interleave x and wv streams
                            nc.sync.dma_start(
                                xt[:, 2 + k, :],
                                xt_d[(2 + k) * 128:(3 + k) * 128, :])
                        if n == 1 and k == 4:
                            emit_const_dmas()
                        if n == 1 and k == 6:
                            emit_qk_load(0)
                            emit_qk_load(1)
                        wvt = wvp.tile([128, 512], bf16)
                        nc.sync.dma_start(
                            wvt[:],
                            wv_d[k * 128:(k + 1) * 128, n * 512:(n + 1) * 512])
                        for i in range(NT):
                            _mm(nc, pss[i][:], xt[:, 2 + k, i * 128:(i + 1) * 128],
                                wvt[:], k == 0, k == NKD - 1)
                    if n == 0:
                        nc.sync.dma_start(bvF[:], bvf_d[:])
                        # preload the Exp activation table before attention
                        nc.scalar.activation(dumt[0:1, 0:1], epsP[0:1, :], AF.Exp)
                    for i in range(NT):
                        dest = vaug[:, i, :].rearrange("p (h c) -> p h c", c=HV)
                        dest = dest[:, n * 8:(n + 1) * 8, 0:DH]
                        vt = vtmp.tile([128, 512], bf16, tag="vt")
                        nc.scalar.activation(vt[:], pss[i][:], AF.Identity)
                        nc.vector.tensor_add(dest, vt[:],
                                             bvF[:, n * 512:(n + 1) * 512])

            # ------------ Q/K projections (feature-major) ------------------
            with tc.tile_pool(name="qps", bufs=2, space="PSUM") as qps:
                for i in range(16):
                    if i + 2 < 16:
                        emit_qk_load(i + 2)
                    if i == 4:
                        for kk in range(2):  # conv feature tiles
                            nc.sync.dma_start(
                                xt[:, kk, :], xt_d[kk * 128:(kk + 1) * 128, :])
                    m = i % 8
                    dst, biasP = (qt, bqP) if i < 8 else (kt, bkP)
                    wt = qk_wts[i]
                    ps = qps.tile([128, 2, 512], f32)
                    for n in range(2):
                        for k in range(NKD):
                            _mm(nc, ps[:, n, :], wt[:, k, :],
                                xt[:, 2 + k, n * 512:(n + 1) * 512],
                                k == 0, k == NKD - 1)
                    nc.vector.tensor_scalar_add(
                        dst[:, m, :], ps.rearrange("p a b -> p (a b)"),
                        biasP[:, m:m + 1])

        # ---------------- attention (single head, lag-1 pipelined) ---------
        with (
            tc.tile_pool(name="upool", bufs=3) as upool,
            tc.tile_pool(name="normp", bufs=2) as normp,
            tc.tile_pool(name="aps", bufs=2, space="PSUM") as apsp,
            tc.tile_pool(name="sps", bufs=2, space="PSUM") as spsp,
        ):
            def scores_exp(h, jt):
                prow = (h % 2) * 64
                ktile = h // 2
                sps = spsp.tile([128, 2, 512], f32, name="sps", tag="sps")
                klhs = kt[prow:prow + 64, ktile, jt * 128:(jt + 1) * 128]
                for c in range(2):
                    _mm(nc, sps[:, c, :], klhs,
                        qt[prow:prow + 64, ktile, c * 512:(c + 1) * 512],
                        True, True)
                u = upool.tile([128, T], bf16, name="u", tag="u")
                nc.scalar.activation(
                    u[:], sps.rearrange("p a b -> p (a b)"), AF.Exp,
                    scale=0.125, bias=mbias[:, jt:jt + 1])
                return u

            def pv(h, jt, u, aps):
                vlhs = vaug[:, jt, h * HV:(h + 1) * HV]
                for c in range(2):
                    _mm(nc, aps[:, c, :], vlhs,
                        u[:, c * 512:(c + 1) * 512], jt == 0, jt == NT - 1)

            def recip_norm(h, aps):
                nt_ = normp.tile([1, T], bf16, name="nt", tag="nt")
                with nc.allow_low_precision(reason="bf16 softmax normalizer"):
                    nc.vector.reciprocal(
                        nt_[:], aps[DH:HV, :, :].rearrange("p a b -> p (a b)"))
                return nt_

            def evac_finalize(h, aps, nt_):
                # replicate 1/norm to 64 rows on PE; evacuate attention rows
                # then scale in place (DVE reads at most one PSUM operand)
                prow = (h % 2) * 64
                ktile = h // 2
                rps = spsp.tile([64, 2, 512], f32, name="rps", tag="sps")
                for c in range(2):
                    nc.tensor.matmul(rps[:, c, :], ones_row[:, 0:64],
                                     nt_[:, c * 512:(c + 1) * 512],
                                     start=True, stop=True)
                nc.vector.tensor_copy(
                    attT[prow:prow + 64, ktile, :],
                    aps[0:DH, :, :].rearrange("p a b -> p (a b)"))
                nc.vector.tensor_mul(
                    attT[prow:prow + 64, ktile, :],
                    attT[prow:prow + 64, ktile, :],
                    rps.rearrange("p a b -> p (a b)"))

            def s_emit(g):
                return scores_exp(g // NT, g % NT)

            # score lookahead runs ACROSS head boundaries so the Act engine's
            # exp stream never drains at a head transition
            us = {0: s_emit(0), 1: s_emit(1)}
            pending = None  # (h, aps, norm_tile) awaiting rps + evacuation
            for h in range(H):
                aps = apsp.tile([HV, 2, 512], f32, name="aps", tag="aps")
                for jt in range(NT):
                    g = h * NT + jt
                    if g + 2 < H * NT:
                        us[g + 2] = s_emit(g + 2)
                    if jt == 2 and pending is not None:
                        evac_finalize(*pending)
                        pending = None
                    pv(h, jt, us.pop(g), aps)
                pending = (h, aps, recip_norm(h, aps))
            evac_finalize(*pending)

        qkp.release()
        vp.release()

        # ---------------- h1pre = concat(conv, att@wo + bo) + x ------------
        with (
            tc.tile_pool(name="convp", bufs=2) as convp,
            tc.tile_pool(name="wop", bufs=3) as wop,
            tc.tile_pool(name="ops", bufs=4, space="PSUM") as opsp,
            tc.tile_pool(name="lnps", bufs=1, space="PSUM") as lnps,
            tc.tile_pool(name="sqp", bufs=3) as sqp,
            tc.tile_pool(name="vecp", bufs=1) as vecp,
        ):
            stat = lnps.tile([128, 16], f32, tag="stat")

            def ln1_k(kb):
                sq = sqp.tile([128, T], bf16, tag="sq")
                nc.vector.tensor_mul(sq[:], h1pre[:, kb, :], h1pre[:, kb, :])
                for tc in range(NT):
                    nc.tensor.matmul(
                        stat[:, tc:tc + 1],
                        h1pre[:, kb, tc * 128:(tc + 1) * 128], ones_col[:],
                        start=kb == 0 and tc == 0,
                        stop=kb == NKE - 1 and tc == NT - 1,
                        skip_group_check=True)
                    nc.tensor.matmul(
                        stat[:, 8 + tc:9 + tc],
                        sq[:, tc * 128:(tc + 1) * 128], ones_col[:],
                        start=False, stop=False, skip_group_check=True)

            # preload the Sqrt act table off the LN1 critical path
            nc.scalar.activation(dumt[0:1, 1:2], epsP[0:1, :], AF.Sqrt)
            for i in range(NSTAGE):
                nc.sync.dma_start(w1stage[i][:], w1r_d[i])

            # depthwise conv (DVE) on the first two feature tiles
            for kb in range(2):
                pad = convp.tile([128, T + 2], bf16, tag="pad")
                nc.gpsimd.memset(pad[:, 0:1], 0.0)
                nc.gpsimd.memset(pad[:, T + 1:T + 2], 0.0)
                nc.vector.tensor_copy(pad[:, 1:T + 1], xt[:, kb, :])
                a1 = convp.tile([128, T], bf16, tag="a1")
                nc.vector.tensor_scalar_mul(a1[:], pad[:, 0:T], cwbc[:, 0:1])
                a2 = convp.tile([128, T], bf16, tag="a2")
                nc.vector.scalar_tensor_tensor(
                    a2[:], pad[:, 1:T + 1], cwbc[:, 1:2], a1[:], OP.mult, OP.add)
                a3 = convp.tile([128, T], bf16, tag="a3")
                nc.vector.scalar_tensor_tensor(
                    a3[:], pad[:, 2:T + 2], cwbc[:, 2:3], a2[:], OP.mult, OP.add)
                nc.vector.tensor_add(h1pre[:, kb, :], a3[:], xt[:, kb, :])
                ln1_k(kb)

            # attention out-projection with residual seeded via identity.
            # Each chain's k=7 step (attT[7] lands last, ~4.5us after the
            # final PV) plus its eviction is deferred by one m so the PE
            # stream never stalls on the last head's finalize.
            opq = []

            def oproj_tail(m, n, wt, ps):
                _mm(nc, ps[:], wt[:, 7, :],
                    attT[:, 7, n * 512:(n + 1) * 512], False, True)
                nc.scalar.activation(
                    h1pre[:, 2 + m, n * 512:(n + 1) * 512], ps[:], AF.Identity,
                    bias=boP[:, m:m + 1])

            for m in range(8):
                wt = wop.tile([128, 8, 128], bf16, tag="wo")
                nc.sync.dma_start(wt[:], wor_d[m])
                for n in range(2):
                    ps = opsp.tile([128, 512], f32)
                    _mm(nc, ps[:], ident[:], xt[:, 2 + m, n * 512:(n + 1) * 512],
                        True, False)
                    for k in range(NKD - 1):
                        _mm(nc, ps[:], wt[:, k, :],
                            attT[:, k, n * 512:(n + 1) * 512], False, False)
                    opq.append((m, n, wt, ps))
                    if len(opq) > 2:
                        oproj_tail(*opq.pop(0))
                if m >= 2:
                    ln1_k(m)  # h1pre[m] complete once chain m-2's tails ran
            while opq:
                oproj_tail(*opq.pop(0))
            for kb in (8, 9):
                ln1_k(kb)

            # LayerNorm 1 statistics: tiny [128, 8] row-major chain, then
            # PE transposes to a [1, T] row and Pool partition-broadcasts.
            muF, rsF = _ln_factors(
                nc, vecp, opsp, stat, 0, 8, seqP, ident, ones_row,
                epsP, "1")
            for kb in range(NKE):
                t1 = sqp.tile([128, T], bf16, tag="t1")
                nc.vector.tensor_sub(t1[:], h1pre[:, kb, :], muF[:])
                t2 = sqp.tile([128, T], bf16, tag="t2")
                nc.vector.tensor_mul(t2[:], t1[:], rsF[:])
                nc.scalar.activation(
                    h1[:, kb, :], t2[:], AF.Identity,
                    bias=beta1P[:, kb:kb + 1], scale=g1P[:, kb:kb + 1])

        xtp.release()
        attp.release()
        h1prep.release()

        # ---------------- FFN1: ffb[m] = relu(h1 @ w1 + b1), all resident --
        outp = tc.alloc_tile_pool(name="outp", bufs=1)
        oacc = outp.tile([128, NKE, T], bf16)
        ffbp = tc.alloc_tile_pool(name="ffbp", bufs=1)
        ffb = ffbp.tile([128, NMF, T], bf16)
        w2ctx = tc.tile_pool(name="w2p", bufs=2)
        w2p = w2ctx.__enter__()
        w2ts = {}

        def load_w2(key):
            t = w2p.tile([128, 40, 128], bf16, tag="w2t")
            nc.sync.dma_start(t[:], w2r_d[key[1]])
            w2ts[key] = t

        with (
            tc.tile_pool(name="w1p", bufs=3) as w1p,
            tc.tile_pool(name="ps1", bufs=3, space="PSUM") as ps1,
        ):
            for mf in range(NMF):
                if mf < NSTAGE:
                    w1t = w1stage[mf]
                else:
                    w1t = w1p.tile([128, 10, 128], bf16, tag="w1t")
                    nc.sync.dma_start(w1t[:], w1r_d[mf])
                if mf == 6:
                    load_w2((0, 0))
                if mf == 24:
                    load_w2((0, 1))
                ps = ps1.tile([128, 2, 512], f32)
                for k in range(NKE):
                    for c in range(2):
                        _mm(nc, ps[:, c, :], w1t[:, k, :],
                            h1[:, k, c * 512:(c + 1) * 512], k == 0, k == NKE - 1)
                nc.scalar.activation(
                    ffb[:, mf, :], ps.rearrange("p a b -> p (a b)"),
                    AF.Relu, bias=b1P[:, mf:mf + 1])

        # -------- FFN2 + LayerNorm 2 + store, pipelined over T-halves ------
        # Each T-half runs the full e-sweep; the finished half's LN2 factors,
        # normalize, transpose and DMA-out overlap the other half's PE sweep.
        with (
            tc.tile_pool(name="ps2", bufs=2, space="PSUM") as ps2,
            tc.tile_pool(name="lnst", bufs=1, space="PSUM") as lnst,
            tc.tile_pool(name="psTp", bufs=2, space="PSUM") as psTp,
            tc.tile_pool(name="sq2p", bufs=3) as sq2p,
            tc.tile_pool(name="vec2p", bufs=1) as vec2p,
            tc.tile_pool(name="obuf", bufs=3) as obuf,
        ):
            stat2a = lnst.tile([128, 16], f32, tag="stat2a")
            stat2b = lnst.tile([128, 16], f32, tag="stat2b")
            stats = [stat2a, stat2b]

            def emit_stats2(half, es, sqs):
                st = stats[half]
                for j in range(4):
                    col = half * 4 + j
                    nc.tensor.matmul(
                        st[:, col:col + 1],
                        oacc[:, es, col * 128:(col + 1) * 128], ones_col[:],
                        start=es == 0 and j == 0, stop=es == NKE - 1 and j == 3,
                        skip_group_check=True)
                    nc.tensor.matmul(
                        st[:, 8 + col:9 + col],
                        sqs[:, j * 128:(j + 1) * 128], ones_col[:],
                        start=False, stop=False, skip_group_check=True)

            def post_half(half):
                muF, rsF = _ln_factors(
                    nc, vec2p, ps2, stats[half], half * 4, 4, seqP, ident,
                    ones_row, epsP, f"2{half}")
                cb = half * 512
                for e in range(NKE):
                    t1 = sq2p.tile([128, 512], bf16, tag="t12")
                    nc.vector.tensor_sub(t1[:], oacc[:, e, cb:cb + 512], muF[:])
                    t2 = sq2p.tile([128, 512], bf16, tag="t22")
                    nc.vector.tensor_mul(t2[:], t1[:], rsF[:])
                    nc.scalar.activation(
                        oacc[:, e, cb:cb + 512], t2[:], AF.Identity,
                        bias=beta2P[:, e:e + 1], scale=g2P[:, e:e + 1])

            def store_tb(tb):
                pts = []
                for piece in range(2):
                    # padded to a full PSUM bank so bufs stay bank-aligned
                    pt = psTp.tile([128, 5, 128], bf16, tag="psT",
                                   padded_shape=[128, 8, 128])
                    for j in range(5):
                        e = piece * 5 + j
                        nc.tensor.matmul(
                            pt[:, j, :], oacc[:, e, tb * 128:(tb + 1) * 128],
                            ident[:], start=True, stop=True, is_transpose=True)
                    pts.append(pt)
                ob = obuf.tile([128, EMB], f32)
                nc.scalar.activation(
                    ob[:, 0:640], pts[0].rearrange("p a b -> p (a b)"),
                    AF.Identity)
                nc.vector.tensor_copy(
                    ob[:, 640:1280], pts[1].rearrange("p a b -> p (a b)"))
                nc.sync.dma_start(out_d[tb * 128:(tb + 1) * 128, :], ob[:])

            pending_stats = None
            for half in range(2):
                cb = half * 512
                for e in range(NKE):
                    nxt = (half, e + 1) if e + 1 < NKE else (half + 1, 0)
                    if nxt[0] < 2 and nxt not in w2ts:
                        load_w2(nxt)
                    w2t = w2ts.pop((half, e))
                    pso = ps2.tile([128, 512], f32, tag="pso")
                    for k in range(NMF):
                        _mm(nc, pso[:], w2t[:, k, :], ffb[:, k, cb:cb + 512],
                            k == 0, k == NMF - 1)
                        if k == 8 and pending_stats is not None:
                            # stats for the previous tile land mid-sweep so
                            # the PE never waits on its DVE epilogue
                            emit_stats2(*pending_stats)
                            pending_stats = None
                        if half == 1 and k == 20:
                            # half-0 post-processing rides under half-1's sweep
                            if e == 0:
                                post_half(0)
                            elif e in (2, 4, 6, 8):
                                store_tb(e // 2 - 1)
                    nc.vector.scalar_tensor_tensor(
                        oacc[:, e, cb:cb + 512], pso[:], b2P[:, e:e + 1],
                        h1[:, e, cb:cb + 512], OP.add, OP.add)
                    sq = sq2p.tile([128, 512], bf16, tag="sq2")
                    nc.vector.tensor_mul(sq[:], oacc[:, e, cb:cb + 512],
                                         oacc[:, e, cb:cb + 512])
                    pending_stats = (half, e, sq)
            emit_stats2(*pending_stats)
            post_half(1)
            for tb in range(4, NT):
                store_tb(tb)

        w2ctx.__exit__(None, None, None)
        ffbp.release()
        outp.release()
        h1p.release()
        constp.release()

    return nc


def _split_matmul_waits(bj: bytes) -> bytes:
    """Walrus codegen allows only one sync-wait on Matmult/DMACopy
    instructions; hoist extra waits onto a preceding EventSemaphore."""
    d = json.loads(bj)
    n = 0
    for f in d["functions"]:
        for blk in f["blocks"]:
            out = []
            for inst in blk["instructions"]:
                si = inst.get("sync_info")
                if (si and si.get("on_wait") and len(si["on_wait"]) >= 2
                        and inst.get("opcode") != "EventSemaphore"):
                    waits = si["on_wait"]
                    for w in waits[:-1]:
                        out.append({
                            "debug": inst.get("debug"),
                            "engine": inst["engine"],
                            "ins": [],
                            "outs": [],
                            "name": f"waitfix_{n}",
                            "opcode": "EventSemaphore",
                            "sync_info": {"on_update": [], "on_wait": [w]},
                        })
                        n += 1
                    si["on_wait"] = waits[-1:]
                out.append(inst)
            blk["instructions"] = out
    return json.dumps(d).encode()


_NC_CACHE = None


def _get_nc():
    global _NC_CACHE
    if _NC_CACHE is None:
        nc = build_nc()
        orig = nc.to_json_bytes
        nc.to_json_bytes = lambda: _split_matmul_waits(orig())
        _NC_CACHE = nc
    return _NC_CACHE


def _prep_core_inputs(x_b, mask_b, seq_b, conv_w, wq, bq, wk, bk, wv, bv, wo, bo,
                      w1, b1, w2, b2, g1, beta1, g2, beta2):
    f = np.float32
    bf = ml_dtypes.bfloat16
    mask_b = np.asarray(mask_b)
    masked = (mask_b != 0).astype(f)  # reference: att_mask != 0 -> -1e9 score
    return {
        "xt": np.ascontiguousarray(x_b.T).astype(bf),
        "wv": np.ascontiguousarray(wv).astype(bf),
        "wqr": np.ascontiguousarray(
            wq.reshape(8, 128, 8, 128).transpose(2, 1, 0, 3)).astype(bf),
        "wkr": np.ascontiguousarray(
            wk.reshape(8, 128, 8, 128).transpose(2, 1, 0, 3)).astype(bf),
        "wor": np.ascontiguousarray(
            wo.reshape(8, 128, 8, 128).transpose(2, 1, 0, 3)).astype(bf),
        "w1r": np.ascontiguousarray(
            w1.reshape(10, 128, 40, 128).transpose(2, 1, 0, 3)).astype(bf),
        "w2r": np.ascontiguousarray(
            w2.reshape(40, 128, 10, 128).transpose(2, 1, 0, 3)).astype(bf),
        "bvf": np.tile(np.asarray(bv, f)[None, :], (128, 1)).astype(bf),
        "mbias": np.ascontiguousarray(
            (MASK_NEG * masked).reshape(8, 128).T.astype(f)),
        "bqp": np.ascontiguousarray(np.asarray(bq, f).reshape(8, 128).T),
        "bkp": np.ascontiguousarray(np.asarray(bk, f).reshape(8, 128).T),
        "bop": np.ascontiguousarray(np.asarray(bo, f).reshape(8, 128).T),
        "b1p": np.ascontiguousarray(np.asarray(b1, f).reshape(40, 128).T),
        "b2p": np.ascontiguousarray(np.asarray(b2, f).reshape(10, 128).T),
        "g1p": np.ascontiguousarray(np.asarray(g1, f).reshape(10, 128).T),
        "beta1p": np.ascontiguousarray(np.asarray(beta1, f).reshape(10, 128).T),
        "g2p": np.ascontiguousarray(np.asarray(g2, f).reshape(10, 128).T),
        "beta2p": np.ascontiguousarray(np.asarray(beta2, f).reshape(10, 128).T),
        "cwbc": np.tile(np.asarray(conv_w, f).reshape(K)[None, :], (128, 1)),
        "seqp": np.ascontiguousarray(np.asarray(seq_b, f).reshape(8, 128).T),
        "onescol": np.ones((128, 1), bf),
        "onesrow": np.ones((1, 128), bf),
        "ident": np.eye(128, dtype=f).astype(bf),
    }


def kernel(x, att_mask, seq_mask, conv_w, wq, bq, wk, bk, wv, bv, wo, bo,
           w1, b1, w2, b2, g1, beta1, g2, beta2, _trace=False):
    from concourse.bass_utils import run_bass_kernel_spmd

    nc = _get_nc()
    x = np.asarray(x, dtype=np.float32)
    in_maps = []
    for b in range(B):
        in_maps.append(_prep_core_inputs(
            x[b], np.asarray(att_mask)[b], np.asarray(seq_mask)[b, :, 0],
            np.asarray(conv_w), np.asarray(wq), np.asarray(bq), np.asarray(wk),
            np.asarray(bk), np.asarray(wv), np.asarray(bv), np.asarray(wo),
            np.asarray(bo), np.asarray(w1), np.asarray(b1), np.asarray(w2),
            np.asarray(b2), np.asarray(g1), np.asarray(beta1), np.asarray(g2),
            np.asarray(beta2)))
    res = run_bass_kernel_spmd(nc, in_maps, list(range(B)), trace=_trace)
    out = np.stack([res.results[i]["out"] for i in range(B)], axis=0)
    if _trace:
        return out, res
    return out


# revision 53
# speedup vs baseline: 1.0011x; 1.0011x over previous
"""Trainium2 Bass/Tile kernel for nn_EncoderLayer (dense transformer block).

Strategy: pure data-parallel over batch (B=8 -> 1 batch element per core, no
collectives). Per core, activations are kept feature-major ([D, T]) in bf16
(same PE matmul rate as fp32r, half the DMA/SBUF traffic, 2x DVE). The key
mask folds into the exp bias (per-partition = per-key) so V needs no masking
and a constant ones column appended to V yields the softmax normalizer for
free from the same P@V matmuls. The attention inner loop is software-
pipelined with the score lookahead running across head boundaries, keeping
the Activation engine's exp stream (the bottleneck of that phase) saturated;
1/norm rows are partition-replicated with PE outer products and applied on
the DVE. h1 stays resident in SBUF (no DRAM round trip). FFN1 output (all 40
row-tiles) stays resident in bf16 so FFN2 accumulates entirely in PSUM with
no SBUF accumulation adds. FFN2/LayerNorm2/transpose/store are pipelined
over T-halves: the finished half's LN2 statistics (computed row-major via
1-column PE matmuls), normalize, PE-transpose to row-major, and DMA-out all
ride under the other half's PE sweep. LayerNorm statistic chains run on
[128, 8] row-major tiles (13x cheaper than [1, T] rows on the DVE).
Startup DMAs are interleaved in consumption order; activation tables (Exp,
Sqrt) are preloaded off the critical path with dummy ops.
"""

import json
import sys

if "/opt/trn_rl_repo" not in sys.path:
    sys.path.insert(0, "/opt/trn_rl_repo")

import numpy as np
import ml_dtypes

import concourse.bass as bass
import concourse.mybir as mybir
import concourse.tile as tile

B, T, CC, DM, H, DH, DFF, K = 8, 1024, 256, 1024, 16, 64, 5120, 3
EMB = CC + DM  # 1280
EPS = 1e-6
f32 = mybir.dt.float32
bf16 = mybir.dt.bfloat16
AF = mybir.ActivationFunctionType
OP = mybir.AluOpType

NT = T // 128          # 8 time tiles
NKE = EMB // 128       # 10 embed k-tiles
NKD = DM // 128        # 8 d_model k-tiles
NMF = DFF // 128       # 40 d_ff tiles
HV = DH + 1            # 65: per-head V columns + normalizer ones column
MASK_NEG = -60000.0    # exp(-60000 + s/8) == 0.0 in f32


def _mm(nc, out, lhsT, rhs, start, stop):
    nc.tensor.matmul(out, lhsT, rhs, start=start, stop=stop)


def _ln_factors(nc, pool, mmpool, stat, c0, ntc, seqP, ident, ones_row,
                epsP, sfx):
    """From row-major PSUM sums stat[:, c0:c0+ntc]=sum(x) and
    stat[:, 8+c0:...]=sum(x^2) (indexed by (t%128, t//128)), produce
    muF/rsF [128, ntc*128] bf16 partition-replicated tiles. Elementwise work
    is on [128, ntc] tiles; rows come from 1-column PE transposes; the
    replication is a PE outer product with ones_row."""
    f32 = mybir.dt.float32
    bf16 = mybir.dt.bfloat16
    AF = mybir.ActivationFunctionType
    OP = mybir.AluOpType
    W = ntc * 128
    mur = pool.tile([128, ntc], f32, tag="mur" + sfx)
    nc.vector.tensor_scalar_mul(mur[:], stat[:, c0:c0 + ntc], 1.0 / EMB)
    mu2r = pool.tile([128, ntc], f32, tag="mu2r" + sfx)
    nc.scalar.activation(mu2r[:], mur[:], AF.Square)
    ex2r = pool.tile([128, ntc], f32, tag="ex2r" + sfx)
    nc.vector.tensor_scalar(ex2r[:], stat[:, 8 + c0:8 + c0 + ntc],
                            1.0 / EMB, EPS, OP.mult, OP.add)
    varr = pool.tile([128, ntc], f32, tag="varr" + sfx)
    nc.vector.tensor_sub(varr[:], ex2r[:], mu2r[:])  # includes +eps
    vrecr = pool.tile([128, ntc], f32, tag="mu2r" + sfx)
    nc.vector.reciprocal(vrecr[:], varr[:])
    rsr = pool.tile([128, ntc], f32, tag="ex2r" + sfx)
    nc.scalar.activation(rsr[:], vrecr[:], AF.Sqrt)
    muB = pool.tile([128, ntc], bf16, tag="muB" + sfx)
    with nc.allow_low_precision(reason="bf16 LN factors"):
        nc.vector.tensor_copy(muB[:], mur[:])
    rsB = pool.tile([128, ntc], bf16, tag="rsB" + sfx)
    with nc.allow_low_precision(reason="bf16 LN factors"):
        nc.vector.tensor_mul(rsB[:], rsr[:], seqP[:, c0:c0 + ntc])
    muF = pool.tile([128, W], bf16, tag="muF" + sfx)
    rsF = pool.tile([128, W], bf16, tag="rsF" + sfx)
    for src_, dst in ((muB, muF), (rsB, rsF)):
        rowp = mmpool.tile([1, W], bf16, tag="lnbc", bufs=2)
        for i in range(ntc):
            nc.tensor.matmul(rowp[:, i * 128:(i + 1) * 128],
                             src_[:, i:i + 1], ident[:],
                             start=True, stop=True, is_transpose=True)
        srow = pool.tile([1, W], bf16, tag="srow" + ("m" if dst is muF else "r") + sfx)
        nc.scalar.activation(srow[:], rowp[:], AF.Identity)
        for c in range(W // 512):
            psb = mmpool.tile([128, 512], f32, tag="lnbc", bufs=2)
            nc.tensor.matmul(psb[:], ones_row[:],
                             srow[:, c * 512:(c + 1) * 512],
                             start=True, stop=True)
            if c % 2 == 0:
                nc.scalar.activation(dst[:, c * 512:(c + 1) * 512], psb[:],
                                     AF.Identity)
            else:
                nc.vector.tensor_copy(dst[:, c * 512:(c + 1) * 512], psb[:])
    return muF, rsF


def build_nc():
    nc = bass.Bass()

    xt_d = nc.declare_dram_parameter("xt", [EMB, T], bf16, isOutput=False)
    wv_d = nc.declare_dram_parameter("wv", [DM, DM], bf16, isOutput=False)
    wqr_d = nc.declare_dram_parameter("wqr", [8, 128, 8, 128], bf16, isOutput=False)
    wkr_d = nc.declare_dram_parameter("wkr", [8, 128, 8, 128], bf16, isOutput=False)
    wor_d = nc.declare_dram_parameter("wor", [8, 128, 8, 128], bf16, isOutput=False)
    w1r_d = nc.declare_dram_parameter("w1r", [40, 128, 10, 128], bf16, isOutput=False)
    w2r_d = nc.declare_dram_parameter("w2r", [10, 128, 40, 128], bf16, isOutput=False)
    bvf_d = nc.declare_dram_parameter("bvf", [128, DM], bf16, isOutput=False)
    mbias_d = nc.declare_dram_parameter("mbias", [128, 8], f32, isOutput=False)
    bqp_d = nc.declare_dram_parameter("bqp", [128, 8], f32, isOutput=False)
    bkp_d = nc.declare_dram_parameter("bkp", [128, 8], f32, isOutput=False)
    bop_d = nc.declare_dram_parameter("bop", [128, 8], f32, isOutput=False)
    b1p_d = nc.declare_dram_parameter("b1p", [128, 40], f32, isOutput=False)
    b2p_d = nc.declare_dram_parameter("b2p", [128, 10], f32, isOutput=False)
    g1p_d = nc.declare_dram_parameter("g1p", [128, 10], f32, isOutput=False)
    beta1p_d = nc.declare_dram_parameter("beta1p", [128, 10], f32, isOutput=False)
    g2p_d = nc.declare_dram_parameter("g2p", [128, 10], f32, isOutput=False)
    beta2p_d = nc.declare_dram_parameter("beta2p", [128, 10], f32, isOutput=False)
    cwbc_d = nc.declare_dram_parameter("cwbc", [128, K], f32, isOutput=False)
    seqp_d = nc.declare_dram_parameter("seqp", [128, 8], f32, isOutput=False)
    onescol_d = nc.declare_dram_parameter("onescol", [128, 1], bf16, isOutput=False)
    onesrow_d = nc.declare_dram_parameter("onesrow", [1, 128], bf16, isOutput=False)
    ident_d = nc.declare_dram_parameter("ident", [128, 128], bf16, isOutput=False)
    out_d = nc.declare_dram_parameter("out", [T, EMB], f32, isOutput=True)

    with tile.TileContext(nc) as tc:
        # ---------------- persistent pools (alloc in reverse-death order) ---
        constp = tc.alloc_tile_pool(name="constp", bufs=1)
        h1p = tc.alloc_tile_pool(name="h1p", bufs=1)
        h1 = h1p.tile([128, NKE, T], bf16)
        h1prep = tc.alloc_tile_pool(name="h1prep", bufs=1)
        h1pre = h1prep.tile([128, NKE, T], bf16)
        attp = tc.alloc_tile_pool(name="attp", bufs=1)
        attT = attp.tile([128, NKD, T], bf16)
        xtp = tc.alloc_tile_pool(name="xtp", bufs=1)
        xt = xtp.tile([128, NKE, T], bf16)

        # const tiles allocated now; DMAs deferred past the startup stream
        bvF = constp.tile([128, DM], bf16)
        mbias = constp.tile([128, 8], f32)
        bqP = constp.tile([128, 8], f32)
        bkP = constp.tile([128, 8], f32)
        boP = constp.tile([128, 8], f32)
        b1P = constp.tile([128, 40], f32)
        b2P = constp.tile([128, 10], f32)
        g1P = constp.tile([128, 10], f32)
        beta1P = constp.tile([128, 10], f32)
        g2P = constp.tile([128, 10], f32)
        beta2P = constp.tile([128, 10], f32)
        cwbc = constp.tile([128, K], f32)
        seqP = constp.tile([128, 8], f32)
        ones_col = constp.tile([128, 1], bf16)
        ones_row = constp.tile([1, 128], bf16)
        ident = constp.tile([128, 128], bf16)
        epsP = constp.tile([128, 1], f32)
        nc.gpsimd.memset(epsP[:], EPS)
        dumt = constp.tile([1, 2], f32)
        NSTAGE = 6
        w1stage = [constp.tile([128, 10, 128], bf16, name=f"w1s{i}")
                   for i in range(NSTAGE)]

        def emit_const_dmas():
            nc.sync.dma_start(mbias[:], mbias_d[:])
            nc.sync.dma_start(bqP[:], bqp_d[:])
            nc.sync.dma_start(bkP[:], bkp_d[:])
            nc.sync.dma_start(boP[:], bop_d[:])
            nc.sync.dma_start(b1P[:], b1p_d[:])
            nc.sync.dma_start(b2P[:], b2p_d[:])
            nc.sync.dma_start(g1P[:], g1p_d[:])
            nc.sync.dma_start(beta1P[:], beta1p_d[:])
            nc.sync.dma_start(g2P[:], g2p_d[:])
            nc.sync.dma_start(beta2P[:], beta2p_d[:])
            nc.sync.dma_start(cwbc[:], cwbc_d[:])
            nc.sync.dma_start(seqP[:], seqp_d[:])
            nc.sync.dma_start(ones_col[:], onescol_d[:])
            nc.sync.dma_start(ones_row[:], onesrow_d[:])
            nc.sync.dma_start(ident[:], ident_d[:])

        vp = tc.alloc_tile_pool(name="vp", bufs=1)
        vaug = vp.tile([128, NT, H * HV], bf16)
        # normalizer ones column (col DH of each head slot)
        ocols = vaug.rearrange("p j (h c) -> p (j h) c", c=HV)[:, :, DH:HV]
        nc.gpsimd.memset(ocols, 1.0)

        qkp = tc.alloc_tile_pool(name="qkp", bufs=1)
        qt = qkp.tile([128, NKD, T], bf16)
        kt = qkp.tile([128, NKD, T], bf16)

        with tc.tile_pool(name="wqp", bufs=3) as wqp:
            qk_wts = []

            def emit_qk_load(i):
                wdram = wqr_d if i < 8 else wkr_d
                wt = wqp.tile([128, 8, 128], bf16, tag="wt")
                nc.sync.dma_start(wt[:], wdram[i % 8])
                qk_wts.append(wt)

            # ------------ V projection (row-major, bias, augmented) --------
            with (
                tc.tile_pool(name="wvp", bufs=4) as wvp,
                tc.tile_pool(name="vtmp", bufs=4) as vtmp,
                tc.tile_pool(name="vps", bufs=8, space="PSUM") as vps,
            ):
                for n in range(2):
                    pss = [vps.tile([128, 512], f32, name=f"vps{i}", tag="vps")
                           for i in range(NT)]
                    for k in range(NKD):
                        if n == 0:  # startup: interleave x and wv streams
                            nc.sync.dma_start(
                                xt[:, 2 + k, :],
                                xt_d[(2 + k) * 128:(3 + k) * 128, :])
                        if n == 1 and k == 4:
                            emit_const_dmas()
                        if n == 1 and k == 6:
                            emit_qk_load(0)
                            emit_qk_load(1)
                        wvt = wvp.tile([128, 512], bf16)
                        nc.sync.dma_start(
                            wvt[:],
                            wv_d[k * 128:(k + 1) * 128, n * 512:(n + 1) * 512])
                        for i in range(NT):
                            _mm(nc, pss[i][:], xt[:, 2 + k, i * 128:(i + 1) * 128],
                                wvt[:], k == 0, k == NKD - 1)
                    if n == 0:
                        nc.sync.dma_start(bvF[:], bvf_d[:])
                        # preload the Exp activation table before attention
                        nc.scalar.activation(dumt[0:1, 0:1], epsP[0:1, :], AF.Exp)
                    for i in range(NT):
                        dest = vaug[:, i, :].rearrange("p (h c) -> p h c", c=HV)
                        dest = dest[:, n * 8:(n + 1) * 8, 0:DH]
                        vt = vtmp.tile([128, 512], bf16, tag="vt")
                        nc.scalar.activation(vt[:], pss[i][:], AF.Identity)
                        nc.vector.tensor_add(dest, vt[:],
                                             bvF[:, n * 512:(n + 1) * 512])

            # ------------ Q/K projections (feature-major) ------------------
            with tc.tile_pool(name="qps", bufs=2, space="PSUM") as qps:
                for i in range(16):
                    if i + 2 < 16:
                        emit_qk_load(i + 2)
                    if i == 4:
                        for kk in range(2):  # conv feature tiles
                            nc.sync.dma_start(
                                xt[:, kk, :], xt_d[kk * 128:(kk + 1) * 128, :])
                    m = i % 8
                    dst, biasP = (qt, bqP) if i < 8 else (kt, bkP)
                    wt = qk_wts[i]
                    ps = qps.tile([128, 2, 512], f32)
                    for n in range(2):
                        for k in range(NKD):
                            _mm(nc, ps[:, n, :], wt[:, k, :],
                                xt[:, 2 + k, n * 512:(n + 1) * 512],
                                k == 0, k == NKD - 1)
                    nc.vector.tensor_scalar_add(
                        dst[:, m, :], ps.rearrange("p a b -> p (a b)"),
                        biasP[:, m:m + 1])

        # ---------------- attention (single head, lag-1 pipelined) ---------
        with (
            tc.tile_pool(name="upool", bufs=3) as upool,
            tc.tile_pool(name="normp", bufs=2) as normp,
            tc.tile_pool(name="aps", bufs=2, space="PSUM") as apsp,
            tc.tile_pool(name="sps", bufs=2, space="PSUM") as spsp,
        ):
            def scores_exp(h, jt):
                prow = (h % 2) * 64
                ktile = h // 2
                sps = spsp.tile([128, 2, 512], f32, name="sps", tag="sps")
                klhs = kt[prow:prow + 64, ktile, jt * 128:(jt + 1) * 128]
                for c in range(2):
                    _mm(nc, sps[:, c, :], klhs,
                        qt[prow:prow + 64, ktile, c * 512:(c + 1) * 512],
                        True, True)
                u = upool.tile([128, T], bf16, name="u", tag="u")
                nc.scalar.activation(
                    u[:], sps.rearrange("p a b -> p (a b)"), AF.Exp,
                    scale=0.125, bias=mbias[:, jt:jt + 1])
                return u

            def pv(h, jt, u, aps):
                vlhs = vaug[:, jt, h * HV:(h + 1) * HV]
                for c in range(2):
                    _mm(nc, aps[:, c, :], vlhs,
                        u[:, c * 512:(c + 1) * 512], jt == 0, jt == NT - 1)

            def recip_norm(h, aps):
                nt_ = normp.tile([1, T], bf16, name="nt", tag="nt")
                with nc.allow_low_precision(reason="bf16 softmax normalizer"):
                    nc.vector.reciprocal(
                        nt_[:], aps[DH:HV, :, :].rearrange("p a b -> p (a b)"))
                return nt_

            def evac_finalize(h, aps, nt_):
                # replicate 1/norm to 64 rows on PE; evacuate attention rows
                # then scale in place (DVE reads at most one PSUM operand)
                prow = (h % 2) * 64
                ktile = h // 2
                rps = spsp.tile([64, 2, 512], f32, name="rps", tag="sps")
                for c in range(2):
                    nc.tensor.matmul(rps[:, c, :], ones_row[:, 0:64],
                                     nt_[:, c * 512:(c + 1) * 512],
                                     start=True, stop=True)
                nc.vector.tensor_copy(
                    attT[prow:prow + 64, ktile, :],
                    aps[0:DH, :, :].rearrange("p a b -> p (a b)"))
                nc.vector.tensor_mul(
                    attT[prow:prow + 64, ktile, :],
                    attT[prow:prow + 64, ktile, :],
                    rps.rearrange("p a b -> p (a b)"))

            def s_emit(g):
                return scores_exp(g // NT, g % NT)

            # score lookahead runs ACROSS head boundaries so the Act engine's
            # exp stream never drains at a head transition
            us = {0: s_emit(0), 1: s_emit(1)}
            pending = None  # (h, aps, norm_tile) awaiting rps + evacuation
            for h in range(H):
                aps = apsp.tile([HV, 2, 512], f32, name="aps", tag="aps")
                for jt in range(NT):
                    g = h * NT + jt
                    if g + 2 < H * NT:
                        us[g + 2] = s_emit(g + 2)
                    if jt == 2 and pending is not None:
                        evac_finalize(*pending)
                        pending = None
                    pv(h, jt, us.pop(g), aps)
                pending = (h, aps, recip_norm(h, aps))
            evac_finalize(*pending)

        qkp.release()
        vp.release()

        # ---------------- h1pre = concat(conv, att@wo + bo) + x ------------
        with (
            tc.tile_pool(name="convp", bufs=2) as convp,
            tc.tile_pool(name="wop", bufs=3) as wop,
            tc.tile_pool(name="ops", bufs=4, space="PSUM") as opsp,
            tc.tile_pool(name="lnps", bufs=1, space="PSUM") as lnps,
            tc.tile_pool(name="sqp", bufs=3) as sqp,
            tc.tile_pool(name="vecp", bufs=1) as vecp,
        ):
            stat = lnps.tile([128, 16], f32, tag="stat")

            def ln1_k(kb):
                sq = sqp.tile([128, T], bf16, tag="sq")
                nc.vector.tensor_mul(sq[:], h1pre[:, kb, :], h1pre[:, kb, :])
                for tc in range(NT):
                    nc.tensor.matmul(
                        stat[:, tc:tc + 1],
                        h1pre[:, kb, tc * 128:(tc + 1) * 128], ones_col[:],
                        start=kb == 0 and tc == 0,
                        stop=kb == NKE - 1 and tc == NT - 1,
                        skip_group_check=True)
                    nc.tensor.matmul(
                        stat[:, 8 + tc:9 + tc],
                        sq[:, tc * 128:(tc + 1) * 128], ones_col[:],
                        start=False, stop=False, skip_group_check=True)

            # preload the Sqrt act table off the LN1 critical path
            nc.scalar.activation(dumt[0:1, 1:2], epsP[0:1, :], AF.Sqrt)
            for i in range(NSTAGE):
                nc.sync.dma_start(w1stage[i][:], w1r_d[i])

            # depthwise conv (DVE) on the first two feature tiles
            for kb in range(2):
                pad = convp.tile([128, T + 2], bf16, tag="pad")
                nc.gpsimd.memset(pad[:, 0:1], 0.0)
                nc.gpsimd.memset(pad[:, T + 1:T + 2], 0.0)
                nc.vector.tensor_copy(pad[:, 1:T + 1], xt[:, kb, :])
                a1 = convp.tile([128, T], bf16, tag="a1")
                nc.vector.tensor_scalar_mul(a1[:], pad[:, 0:T], cwbc[:, 0:1])
                a2 = convp.tile([128, T], bf16, tag="a2")
                nc.vector.scalar_tensor_tensor(
                    a2[:], pad[:, 1:T + 1], cwbc[:, 1:2], a1[:], OP.mult, OP.add)
                a3 = convp.tile([128, T], bf16, tag="a3")
                nc.vector.scalar_tensor_tensor(
                    a3[:], pad[:, 2:T + 2], cwbc[:, 2:3], a2[:], OP.mult, OP.add)
                nc.vector.tensor_add(h1pre[:, kb, :], a3[:], xt[:, kb, :])
                ln1_k(kb)

            # attention out-projection with residual seeded via identity
            for m in range(8):
                wt = wop.tile([128, 8, 128], bf16, tag="wo")
                nc.sync.dma_start(wt[:], wor_d[m])
                for n in range(2):
                    ps = opsp.tile([128, 512], f32)
                    _mm(nc, ps[:], ident[:], xt[:, 2 + m, n * 512:(n + 1) * 512],
                        True, False)
                    for k in range(NKD):
                        _mm(nc, ps[:], wt[:, k, :],
                            attT[:, k, n * 512:(n + 1) * 512], False, k == NKD - 1)
                    nc.scalar.activation(
                        h1pre[:, 2 + m, n * 512:(n + 1) * 512], ps[:], AF.Identity,
                        bias=boP[:, m:m + 1])
                ln1_k(2 + m)

            # LayerNorm 1 statistics: tiny [128, 8] row-major chain, then
            # PE transposes to a [1, T] row and Pool partition-broadcasts.
            muF, rsF = _ln_factors(
                nc, vecp, opsp, stat, 0, 8, seqP, ident, ones_row,
                epsP, "1")
            for kb in range(NKE):
                t1 = sqp.tile([128, T], bf16, tag="t1")
                nc.vector.tensor_sub(t1[:], h1pre[:, kb, :], muF[:])
                t2 = sqp.tile([128, T], bf16, tag="t2")
                nc.vector.tensor_mul(t2[:], t1[:], rsF[:])
                nc.scalar.activation(
                    h1[:, kb, :], t2[:], AF.Identity,
                    bias=beta1P[:, kb:kb + 1], scale=g1P[:, kb:kb + 1])

        xtp.release()
        attp.release()
        h1prep.release()

        # ---------------- FFN1: ffb[m] = relu(h1 @ w1 + b1), all resident --
        outp = tc.alloc_tile_pool(name="outp", bufs=1)
        oacc = outp.tile([128, NKE, T], bf16)
        ffbp = tc.alloc_tile_pool(name="ffbp", bufs=1)
        ffb = ffbp.tile([128, NMF, T], bf16)
        w2ctx = tc.tile_pool(name="w2p", bufs=2)
        w2p = w2ctx.__enter__()
        w2ts = {}

        def load_w2(key):
            t = w2p.tile([128, 40, 128], bf16, tag="w2t")
            nc.sync.dma_start(t[:], w2r_d[key[1]])
            w2ts[key] = t

        with (
            tc.tile_pool(name="w1p", bufs=3) as w1p,
            tc.tile_pool(name="ps1", bufs=3, space="PSUM") as ps1,
        ):
            for mf in range(NMF):
                if mf < NSTAGE:
                    w1t = w1stage[mf]
                else:
                    w1t = w1p.tile([128, 10, 128], bf16, tag="w1t")
                    nc.sync.dma_start(w1t[:], w1r_d[mf])
                if mf == 6:
                    load_w2((0, 0))
                if mf == 24:
                    load_w2((0, 1))
                ps = ps1.tile([128, 2, 512], f32)
                for k in range(NKE):
                    for c in range(2):
                        _mm(nc, ps[:, c, :], w1t[:, k, :],
                            h1[:, k, c * 512:(c + 1) * 512], k == 0, k == NKE - 1)
                nc.scalar.activation(
                    ffb[:, mf, :], ps.rearrange("p a b -> p (a b)"),
                    AF.Relu, bias=b1P[:, mf:mf + 1])

        # -------- FFN2 + LayerNorm 2 + store, pipelined over T-halves ------
        # Each T-half runs the full e-sweep; the finished half's LN2 factors,
        # normalize, transpose and DMA-out overlap the other half's PE sweep.
        with (
            tc.tile_pool(name="ps2", bufs=2, space="PSUM") as ps2,
            tc.tile_pool(name="lnst", bufs=1, space="PSUM") as lnst,
            tc.tile_pool(name="psTp", bufs=2, space="PSUM") as psTp,
            tc.tile_pool(name="sq2p", bufs=3) as sq2p,
            tc.tile_pool(name="vec2p", bufs=1) as vec2p,
            tc.tile_pool(name="obuf", bufs=3) as obuf,
        ):
            stat2a = lnst.tile([128, 16], f32, tag="stat2a")
            stat2b = lnst.tile([128, 16], f32, tag="stat2b")
            stats = [stat2a, stat2b]

            def emit_stats2(half, es, sqs):
                st = stats[half]
                for j in range(4):
                    col = half * 4 + j
                    nc.tensor.matmul(
                        st[:, col:col + 1],
                        oacc[:, es, col * 128:(col + 1) * 128], ones_col[:],
                        start=es == 0 and j == 0, stop=es == NKE - 1 and j == 3,
                        skip_group_check=True)
                    nc.tensor.matmul(
                        st[:, 8 + col:9 + col],
                        sqs[:, j * 128:(j + 1) * 128], ones_col[:],
                        start=False, stop=False, skip_group_check=True)

            def post_half(half):
                muF, rsF = _ln_factors(
                    nc, vec2p, ps2, stats[half], half * 4, 4, seqP, ident,
                    ones_row, epsP, f"2{half}")
                cb = half * 512
                for e in range(NKE):
                    t1 = sq2p.tile([128, 512], bf16, tag="t12")
                    nc.vector.tensor_sub(t1[:], oacc[:, e, cb:cb + 512], muF[:])
                    t2 = sq2p.tile([128, 512], bf16, tag="t22")
                    nc.vector.tensor_mul(t2[:], t1[:], rsF[:])
                    nc.scalar.activation(
                        oacc[:, e, cb:cb + 512], t2[:], AF.Identity,
                        bias=beta2P[:, e:e + 1], scale=g2P[:, e:e + 1])

            def store_tb(tb):
                pts = []
                for piece in range(2):
                    # padded to a full PSUM bank so bufs stay bank-aligned
                    pt = psTp.tile([128, 5, 128], bf16, tag="psT",
                                   padded_shape=[128, 8, 128])
                    for j in range(5):
                        e = piece * 5 + j
                        nc.tensor.matmul(
                            pt[:, j, :], oacc[:, e, tb * 128:(tb + 1) * 128],
                            ident[:], start=True, stop=True, is_transpose=True)
                    pts.append(pt)
                ob = obuf.tile([128, EMB], f32)
                nc.scalar.activation(
                    ob[:, 0:640], pts[0].rearrange("p a b -> p (a b)"),
                    AF.Identity)
                nc.vector.tensor_copy(
                    ob[:, 640:1280], pts[1].rearrange("p a b -> p (a b)"))
                nc.sync.dma_start(out_d[tb * 128:(tb + 1) * 128, :], ob[:])

            pending_stats = None
            for half in range(2):
                cb = half * 512
                for e in range(NKE):
                    nxt = (half, e + 1) if e + 1 < NKE else (half + 1, 0)
                    if nxt[0] < 2 and nxt not in w2ts:
                        load_w2(nxt)
                    w2t = w2ts.pop((half, e))
                    pso = ps2.tile([128, 512], f32, tag="pso")
                    for k in range(NMF):
                        _mm(nc, pso[:], w2t[:, k, :], ffb[:, k, cb:cb + 512],
                            k == 0, k == NMF - 1)
                        if k == 8 and pending_stats is not None:
                            # stats for the previous tile land mid-sweep so
                            # the PE never waits on its DVE epilogue
                            emit_stats2(*pending_stats)
                            pending_stats = None
                        if half == 1 and k == 20:
                            # half-0 post-processing rides under half-1's sweep
                            if e == 0:
                                post_half(0)
                            elif e in (2, 4, 6, 8):
                                store_tb(e // 2 - 1)
                    nc.vector.scalar_tensor_tensor(
                        oacc[:, e, cb:cb + 512], pso[:], b2P[:, e:e + 1],
                        h1[:, e, cb:cb + 512], OP.add, OP.add)
                    sq = sq2p.tile([128, 512], bf16, tag="sq2")
                    nc.vector.tensor_mul(sq[:], oacc[:, e, cb:cb + 512],
                                         oacc[:, e, cb:cb + 512])
                    pending_stats = (half, e, sq)
            emit_stats2(*pending_stats)
            post_half(1)
            for tb in range(4, NT):
                store_tb(tb)

        w2ctx.__exit__(None, None, None)
        ffbp.release()
        outp.release()
        h1p.release()
        constp.release()

    return nc


def _split_matmul_waits(bj: bytes) -> bytes:
    """Walrus codegen allows only one sync-wait on Matmult/DMACopy
    instructions; hoist extra waits onto a preceding EventSemaphore."""
    d = json.loads(bj)
    n = 0
    for f in d["functions"]:
        for blk in f["blocks"]:
            out = []
            for inst in blk["instructions"]:
                si = inst.get("sync_info")
                if (si and si.get("on_wait") and len(si["on_wait"]) >= 2
                        and inst.get("opcode") != "EventSemaphore"):
                    waits = si["on_wait"]
                    for w in waits[:-1]:
                        out.append({
                            "debug": inst.get("debug"),
                            "engine": inst["engine"],
                            "ins": [],
                            "outs": [],
                            "name": f"waitfix_{n}",
                            "opcode": "EventSemaphore",
                            "sync_info": {"on_update": [], "on_wait": [w]},
                        })
                        n += 1
                    si["on_wait"] = waits[-1:]
                out.append(inst)
            blk["instructions"] = out
    return json.dumps(d).encode()


_NC_CACHE = None


def _get_nc():
    global _NC_CACHE
    if _NC_CACHE is None:
        nc = build_nc()
        orig = nc.to_json_bytes
        nc.to_json_bytes = lambda: _split_matmul_waits(orig())
        _NC_CACHE = nc
    return _NC_CACHE


def _prep_core_inputs(x_b, mask_b, seq_b, conv_w, wq, bq, wk, bk, wv, bv, wo, bo,
                      w1, b1, w2, b2, g1, beta1, g2, beta2):
    f = np.float32
    bf = ml_dtypes.bfloat16
    mask_b = np.asarray(mask_b)
    masked = (mask_b != 0).astype(f)  # reference: att_mask != 0 -> -1e9 score
    return {
        "xt": np.ascontiguousarray(x_b.T).astype(bf),
        "wv": np.ascontiguousarray(wv).astype(bf),
        "wqr": np.ascontiguousarray(
            wq.reshape(8, 128, 8, 128).transpose(2, 1, 0, 3)).astype(bf),
        "wkr": np.ascontiguousarray(
            wk.reshape(8, 128, 8, 128).transpose(2, 1, 0, 3)).astype(bf),
        "wor": np.ascontiguousarray(
            wo.reshape(8, 128, 8, 128).transpose(2, 1, 0, 3)).astype(bf),
        "w1r": np.ascontiguousarray(
            w1.reshape(10, 128, 40, 128).transpose(2, 1, 0, 3)).astype(bf),
        "w2r": np.ascontiguousarray(
            w2.reshape(40, 128, 10, 128).transpose(2, 1, 0, 3)).astype(bf),
        "bvf": np.tile(np.asarray(bv, f)[None, :], (128, 1)).astype(bf),
        "mbias": np.ascontiguousarray(
            (MASK_NEG * masked).reshape(8, 128).T.astype(f)),
        "bqp": np.ascontiguousarray(np.asarray(bq, f).reshape(8, 128).T),
        "bkp": np.ascontiguousarray(np.asarray(bk, f).reshape(8, 128).T),
        "bop": np.ascontiguousarray(np.asarray(bo, f).reshape(8, 128).T),
        "b1p": np.ascontiguousarray(np.asarray(b1, f).reshape(40, 128).T),
        "b2p": np.ascontiguousarray(np.asarray(b2, f).reshape(10, 128).T),
        "g1p": np.ascontiguousarray(np.asarray(g1, f).reshape(10, 128).T),
        "beta1p": np.ascontiguousarray(np.asarray(beta1, f).reshape(10, 128).T),
        "g2p": np.ascontiguousarray(np.asarray(g2, f).reshape(10, 128).T),
        "beta2p": np.ascontiguousarray(np.asarray(beta2, f).reshape(10, 128).T),
        "cwbc": np.tile(np.asarray(conv_w, f).reshape(K)[None, :], (128, 1)),
        "seqp": np.ascontiguousarray(np.asarray(seq_b, f).reshape(8, 128).T),
        "onescol": np.ones((128, 1), bf),
        "onesrow": np.ones((1, 128), bf),
        "ident": np.eye(128, dtype=f).astype(bf),
    }


def kernel(x, att_mask, seq_mask, conv_w, wq, bq, wk, bk, wv, bv, wo, bo,
           w1, b1, w2, b2, g1, beta1, g2, beta2, _trace=False):
    from concourse.bass_utils import run_bass_kernel_spmd

    nc = _get_nc()
    x = np.asarray(x, dtype=np.float32)
    in_maps = []
    for b in range(B):
        in_maps.append(_prep_core_inputs(
            x[b], np.asarray(att_mask)[b], np.asarray(seq_mask)[b, :, 0],
            np.asarray(conv_w), np.asarray(wq), np.asarray(bq), np.asarray(wk),
            np.asarray(bk), np.asarray(wv), np.asarray(bv), np.asarray(wo),
            np.asarray(bo), np.asarray(w1), np.asarray(b1), np.asarray(w2),
            np.asarray(b2), np.asarray(g1), np.asarray(beta1), np.asarray(g2),
            np.asarray(beta2)))
    res = run_bass_kernel_spmd(nc, in_maps, list(range(B)), trace=_trace)
    out = np.stack([res.results[i]["out"] for i in range(B)], axis=0)
    if _trace:
        return out, res
    return out
